# revision 25
# baseline (speedup 1.0000x reference)
"""Trainium2 Bass kernel for nn_BertPooler (binarized BertPooler head).

Math (see reference):
    x   = hidden_states[:, 0, :]                      # [B, H] first token
    xq  = sign(x) * max(alpha, 1e-5)
    wq  = sign(W) * mean(|W|)
    y   = tanh(xq @ wq.T + b)                         # [B, 1, H]

Sharding (8 cores):
  - Output features o are sharded 128 per core. Core c computes
    y[:, 0, 128c:128c+128].
  - Each core receives the FULL weight matrix (rolled so that its own
    128 output rows come first) because mean(|W|) is a global reduction;
    the 4 MB W load is the per-core memory roofline.
  - hidden_states is sliced to the first token on the host (pure data
    movement); the 128 MB bulk tensor is never touched by the device.

Per-core device program:
  - Small inputs (x^T, bias, alpha) DMA on the scalar HWDGE ring so they
    are not queued behind the 4 MB W load on the sync ring.
  - W arrives in 5 chunks (0.5/1/1/1/0.5 MB); DVE abs-reduces each as it
    lands (-> mean|W|). Small first chunk = the matmul shard (early
    sign/transpose); small last chunk shortens the tail reduce.
  - ACT sign of shard + x^T (bf16), 8 PE transposes, 1 big PSUM->SBUF
    copy, 8 accumulating PE matmuls: S[o,b] = sum_h sg(W)[o,h] sg(x)[b,h].
  - Partition-broadcast of (sum|W|, clamped alpha) via a ones-matmul,
    ordered after the main matmuls so it doesn't stall them.
  - One ACT instruction: y = tanh(S * (alpha*mean|W|) + b_shard).
All arithmetic of the reference runs on device; the host only
slices/permutes inputs and reassembles the output.
"""

import os
import sys

import numpy as np

sys.path.insert(0, "/opt/trn_rl_repo")

import concourse.bass as bass  # noqa: E402
import concourse.mybir as mybir  # noqa: E402
from concourse import bacc  # noqa: E402
from concourse.bass_utils import run_bass_kernel_spmd  # noqa: E402
from concourse.masks import make_identity  # noqa: E402
from concourse.tile import TileContext  # noqa: E402
from concourse.tile_rust import add_dep_helper  # noqa: E402


def _ensure_axon_ntff_hook():
    """Register the axon NTFF profiling hook if the image's antenv lacks
    the antenv.axon_hooks registration channel. Without this, running
    with BASS_TRACE=1 raises ModuleNotFoundError in bass_utils; with it,
    tracing works (or degrades gracefully if the .so is too old)."""
    try:
        import antenv.axon_hooks  # noqa: F401

        return
    except ImportError:
        pass
    try:
        import types

        import antenv

        mod = types.ModuleType("antenv.axon_hooks")
        mod._hook = None

        def set_axon_ntff_profile_hook(h):
            mod._hook = h

        def get_axon_ntff_profile_hook():
            return mod._hook

        mod.set_axon_ntff_profile_hook = set_axon_ntff_profile_hook
        mod.get_axon_ntff_profile_hook = get_axon_ntff_profile_hook
        sys.modules["antenv.axon_hooks"] = mod
        antenv.axon_hooks = mod

        from trn_agent_boot.trn_boot import _ntff_profile_via_ctypes

        so_path = "/opt/axon/libaxon_pjrt.so"
        if os.path.exists(so_path):
            hook = _ntff_profile_via_ctypes(so_path)
            if hook is not None:
                set_axon_ntff_profile_hook(hook)
    except Exception:
        pass


_ensure_axon_ntff_hook()

B, S, H = 8, 4096, 1024
NCORES = 8
OSH = H // NCORES  # 128 output features per core
EPS = 1e-5

_NC = None
LAST_RESULTS = None


def _raw(inst):
    return getattr(inst, "ins", inst)


def _build():
    # Bacc (not plain Bass): its compile() pass pipeline splits multi-sem
    # waits into event semaphores — TRN2 allows only 1 wait per instruction.
    nc = bacc.Bacc(None, enable_partition_id=False)
    f32 = mybir.dt.float32
    bf16 = mybir.dt.bfloat16

    # Wsm: the core's shard rows of the rolled W with the small operands
    # concatenated as extra columns (cols 0:1024 = W rows, 1024:1088 =
    # x^T layout, col 1088 = bias shard, col 1089 = alpha replicated).
    # One contiguous 4360 B/partition DMA — no fine-grained side traffic.
    Wsm = nc.dram_tensor("Wsm", [128, H + 66], f32, kind="ExternalInput")
    # Wrest: the remaining 896 rows of the rolled W.
    Wrest = nc.dram_tensor("Wrest", [H - OSH, H], f32, kind="ExternalInput")
    yT = nc.dram_tensor("yT", [OSH, B], f32, kind="ExternalOutput")

    with TileContext(nc) as tc:
        with (
            tc.tile_pool(name="w", bufs=6) as wpool,
            tc.tile_pool(name="s", bufs=1) as spool,
            tc.tile_pool(name="ptp", bufs=1, space="PSUM") as ptp,
            tc.tile_pool(name="pacc", bufs=1, space="PSUM") as pacc,
        ):
            # ---- W load: 6 chunks. First carries shard + small operands.
            # Rows 896-1023 are split into two column halves so the tail
            # abs-reduce is short, with the last on a different engine. ----
            wsh = wpool.tile([128, H + 66], f32, tag="wsh")
            nc.sync.dma_start(out=wsh[:], in_=Wsm[:])
            smt = wsh[:, H : H + 66]
            wmids = []
            for t in range(3):
                wm = wpool.tile([128, 2, 1024], f32, tag="wmid")
                src = Wrest[256 * t : 256 * (t + 1), :].rearrange(
                    "(two p) h -> p two h", p=128
                )
                nc.sync.dma_start(out=wm[:], in_=src)
                wmids.append(wm)
            wl0 = wpool.tile([128, 512], f32, tag="wl0")
            nc.sync.dma_start(out=wl0[:], in_=Wrest[768:896, 0:512])
            wl1 = wpool.tile([128, 512], f32, tag="wl1")
            nc.sync.dma_start(out=wl1[:], in_=Wrest[768:896, 512:1024])

            # ---- identity for PE transpose, built on-chip (no DMA) ----
            idt = spool.tile([128, 128], bf16)
            make_identity(nc, idt[:])

            # ---- sign of x^T and of this core's W shard (bf16) ----
            sx = spool.tile([128, 64], bf16)
            nc.scalar.activation(
                sx[:], smt[:, 0:64], mybir.ActivationFunctionType.Sign
            )
            sw = spool.tile([128, 1024], bf16)
            nc.scalar.activation(
                sw[:], wsh[:, 0:H], mybir.ActivationFunctionType.Sign
            )

            # ---- abs partial sums, one col per chunk ----
            parts = spool.tile([128, 6], f32)
            nc.vector.tensor_reduce(
                out=parts[:, 0:1],
                in_=wsh[:, 0:H],
                axis=mybir.AxisListType.X,
                op=mybir.AluOpType.add,
                apply_absolute_value=True,
            )
            for t in range(3):
                nc.vector.tensor_reduce(
                    out=parts[:, 1 + t : 2 + t],
                    in_=wmids[t][:],
                    axis=mybir.AxisListType.XY,
                    op=mybir.AluOpType.add,
                    apply_absolute_value=True,
                )
            nc.vector.tensor_reduce(
                out=parts[:, 4:5],
                in_=wl0[:],
                axis=mybir.AxisListType.X,
                op=mybir.AluOpType.add,
                apply_absolute_value=True,
            )
            # last tail chunk reduced on the Scalar engine (Abs+accum) so
            # the two tail reduces run in parallel
            wl1_abs = spool.tile([128, 512], f32)
            nc.scalar.activation(
                wl1_abs[:],
                wl1[:],
                mybir.ActivationFunctionType.Abs,
                accum_out=parts[:, 5:6],
            )

            # ---- alpha clamp (already per-partition) + total abs sum ----
            alc = spool.tile([128, 1], f32)
            nc.vector.tensor_scalar_max(alc[:], smt[:, 65:66], EPS)
            rhs_bc = spool.tile([128, 1], f32)
            nc.vector.tensor_reduce(
                out=rhs_bc[:, 0:1],
                in_=parts[:],
                axis=mybir.AxisListType.X,
                op=mybir.AluOpType.add,
            )

            # ---- transpose shard blocks: sw [o,h] -> swt chunks [h,o] ----
            tp_all = ptp.tile([128, 8, 128], bf16)  # one PSUM bank
            for hc in range(8):
                nc.tensor.transpose(
                    tp_all[:, hc, :], sw[:, 128 * hc : 128 * (hc + 1)], idt[:]
                )
            swt_all = spool.tile([128, 8, 128], bf16)
            nc.vector.tensor_copy(swt_all[:], tp_all[:])

            # ---- S[o, b] = sum_h sign(W)[o, h] * sign(x)[b, h] ----
            s_ps = pacc.tile([128, B], f32)
            mm_last = None
            for hc in range(8):
                mm_last = nc.tensor.matmul(
                    s_ps[:],
                    swt_all[:, hc, :],
                    sx[:, B * hc : B * (hc + 1)],
                    start=(hc == 0),
                    stop=(hc == 7),
                )

            # ---- broadcast sum|W| to all partitions via ones-matmul ----
            ones = spool.tile([128, 128], f32)
            nc.vector.memset(ones[:], 1.0)
            bc_ps = pacc.tile([128, 1], f32)
            bc_mm = nc.tensor.matmul(bc_ps[:], ones[:], rhs_bc[:], start=True, stop=True)
            # The bcast matmul is only ready after the full |W| reduction;
            # keep it behind the early-ready main matmuls in PE order.
            add_dep_helper(
                _raw(bc_mm), _raw(mm_last), sync=False, reason="bc after mms"
            )

            # scale = alpha_c * sum|W| / (H*H)
            scale = spool.tile([128, 1], f32)
            nc.vector.tensor_scalar(
                out=scale[:],
                in0=bc_ps[:, 0:1],
                scalar1=alc[:],
                scalar2=1.0 / (H * H),
                op0=mybir.AluOpType.mult,
                op1=mybir.AluOpType.mult,
            )

            # ---- y^T = tanh(S * scale + b), one ACT instruction;
            # output DMA issued from the same engine (no extra sem hop) ----
            ysb = spool.tile([OSH, B], f32)
            nc.scalar.activation(
                ysb[:],
                s_ps[:],
                mybir.ActivationFunctionType.Tanh,
                bias=smt[:, 64:65],
                scale=scale[:],
            )
            nc.scalar.dma_start(out=yT[:], in_=ysb[:])

    nc.compile()
    return nc


def _get_nc():
    global _NC
    if _NC is None:
        _NC = _build()
    return _NC


def kernel(hidden_states, W, b, alpha):
    global LAST_RESULTS
    hidden_states = np.asarray(hidden_states, dtype=np.float32)
    W = np.asarray(W, dtype=np.float32)
    b = np.asarray(b, dtype=np.float32)
    alpha = np.asarray(alpha, dtype=np.float32)

    # Host-side data movement only: slice first token, transpose layout,
    # pack shard + small operands into one contiguous tensor per core.
    x = np.ascontiguousarray(hidden_states[:, 0, :])  # [B, H]
    # xTl[p, hc*8 + b] = x[b, hc*128 + p]
    xTl = x.reshape(B, 8, 128).transpose(2, 1, 0).reshape(128, 64)

    in_maps = []
    for c in range(NCORES):
        rows = np.roll(W, -OSH * c, axis=0)
        Wsm = np.empty((OSH, H + 66), dtype=np.float32)
        Wsm[:, 0:H] = rows[0:OSH]
        Wsm[:, H : H + 64] = xTl
        Wsm[:, H + 64] = b[OSH * c : OSH * (c + 1)]
        Wsm[:, H + 65] = alpha[0]
        in_maps.append(
            {"Wsm": Wsm, "Wrest": np.ascontiguousarray(rows[OSH:])}
        )

    nc = _get_nc()
    res = run_bass_kernel_spmd(nc, in_maps, core_ids=list(range(NCORES)))
    LAST_RESULTS = res

    out = np.empty((B, 1, H), dtype=np.float32)
    for c in range(NCORES):
        out[:, 0, OSH * c : OSH * (c + 1)] = res.results[c]["yT"].T
    return out
